# revision 38
# baseline (speedup 1.0000x reference)
"""Trainium2 Bass kernel: fused MHA block (LN -> QKV -> q/k per-token LN ->
RoPE -> SDPA -> out-proj), SPMD over 8 NeuronCores.

Sharding (v3): core c handles batch b = c//4 and token quarter s = c%4 in
GLOBAL token order. Each core projects q/k/v only for its own 512-token
quarter (no cross-core redundancy), then the 4 cores of a batch AllGather
the roped keys and the values; every core runs attention for its 512
queries over all 2048 keys. Host concatenates 8 [512, 1024] output slices.

Design notes (all matmuls bf16 with f32 PSUM accumulation):
  - ln_w folded into w_qkv on the host (W' = W * ln_w); nonzero ln_b enters
    as one K=1 ones-matmul accumulate (c0 = W @ ln_b) per projection half.
  - x normalized IN PLACE in the e-major x slab tile: stats via ones(1/D)
    matmuls; r and mu*r rows broadcast across partitions with K=1 matmuls
    into PSUM (no DRAM bounce).
  - per-token q/k LN: bn_stats on DVE, affine applied on ACT via per-token
    scale/bias pointers (in place); RoPE sin-mul on DVE, cos-mul + add on
    Pool.
  - AllGather #1: roped k quarter [512, D] -> [2048, D], then 4 XBAR
    transposes into feature-major krT. AllGather #2: v quarter -> v_sb
    (strided loads add the fused softmax-denominator ones column).
  - attention: scoresT = krT.T @ qrT per head pair, exp on ACT (the binding
    engine); AV ones-column produces the softmax denominator; denominator
    reciprocal broadcast via K=1 matmul. PSUM evacuations on DVE so ACT
    does nothing but exp.
  - transposes (XBAR) from SP, weight DMAs split gpsimd/sync.
"""

import numpy as np
import ml_dtypes

import concourse.bass as bass
import concourse.mybir as mybir
import concourse.tile as tile
from concourse import bacc
from concourse.bass_utils import run_bass_kernel_spmd

B, L, D, H, DH = 2, 2048, 1024, 16, 64
EPS = 1e-5
ROPE_BASE = 10000.0
NCORES = 8
LQ = L // 4
P = 128
ND = D // P      # 8 feature tiles of 128
NT = L // P      # 16 key token tiles
NTQ = LQ // P    # 4 local token tiles
FD = 512         # psum bank free size (f32)
NSL = L // FD    # 4 key slabs of 512
BF = mybir.dt.bfloat16
F32 = mybir.dt.float32
AF = mybir.ActivationFunctionType
OP = mybir.AluOpType
GROUPS = [[0, 1, 2, 3], [4, 5, 6, 7]]


def _bc_part(ap, parts):
    """Partition-broadcast (step 0) of a [1, ...] DRAM AP to `parts` rows."""
    return bass.AP(tensor=ap.tensor, offset=ap.offset,
                   ap=[[0, parts]] + list(ap.ap[1:]))


def _bc_heads(ap2, n, at=1):
    """Insert a step-0 dim of size n at free position `at` of a 2D sbuf AP."""
    dims = list(ap2.ap)
    return bass.AP(tensor=ap2.tensor, offset=ap2.offset,
                   ap=dims[:at] + [[0, n]] + dims[at:])


def _emit(nc, with_c0):
    xT = nc.dram_tensor("xT", [D, LQ], BF, kind="ExternalInput")
    wqkvT = nc.dram_tensor("wqkvT", [D, 3 * D], BF, kind="ExternalInput")
    woutT = nc.dram_tensor("woutT", [D, D], BF, kind="ExternalInput")
    c0_t = (nc.dram_tensor("c0_t", [3, D], BF, kind="ExternalInput")
            if with_c0 else None)
    srow_t = (None if with_c0 else
              nc.dram_tensor("srow_t", [3, D], BF, kind="ExternalInput"))
    q_ln_w = nc.dram_tensor("q_ln_w", [D], BF, kind="ExternalInput")
    k_ln_w = nc.dram_tensor("k_ln_w", [D], BF, kind="ExternalInput")
    cos_t = nc.dram_tensor("cos_t", [LQ, DH], BF, kind="ExternalInput")
    sin_t = nc.dram_tensor("sin_t", [LQ, DH], BF, kind="ExternalInput")
    out = nc.dram_tensor("out", [LQ, D], F32, kind="ExternalOutput")

    with tile.TileContext(nc) as tc:
        _body(nc, tc, xT, wqkvT, woutT, c0_t, srow_t, q_ln_w, k_ln_w,
              cos_t, sin_t, out)
    return nc


def _rstd_refine(nc, pool, r, vareps, shape, name):
    """One Newton step for r ~= rsqrt(varep): r' = r*(1.5 - 0.5*varep*r^2).
    Guards against ACT sqrt LUT error on hardware. In-place on r."""
    t = pool.tile(list(shape), F32, name=f"{name}_nt", tag=f"{name}_nt", bufs=2)
    nc.scalar.activation(t[:], r[:], AF.Square)
    nc.vector.tensor_mul(t[:], t[:], vareps[:])
    nc.vector.tensor_scalar(t[:], t[:], -0.5, 1.5, op0=OP.mult, op1=OP.add)
    nc.vector.tensor_mul(r[:], r[:], t[:])


def _body(nc, tc, xT, wqkvT, woutT, c0_t, srow_t, q_ln_w, k_ln_w,
          cos_t, sin_t, out):
    import contextlib
    ap_xT = xT.ap().rearrange("(nd p) t -> p nd t", p=P)
    ap_wqkvT = wqkvT.ap().rearrange("(nd p) e -> p nd e", p=P)
    ap_woutT = woutT.ap().rearrange("(nd p) e -> p nd e", p=P)
    ap_cos = cos_t.ap().rearrange("(tt p) j -> p tt j", p=P)
    ap_sin = sin_t.ap().rearrange("(tt p) j -> p tt j", p=P)

    ctx = contextlib.ExitStack()
    with ctx:
        const = ctx.enter_context(tc.tile_pool(name="const", bufs=1))
        wpool = ctx.enter_context(tc.tile_pool(name="wp", bufs=1))
        live = ctx.enter_context(tc.tile_pool(name="live", bufs=1))
        stat = ctx.enter_context(tc.tile_pool(name="stat", bufs=1))
        dram = ctx.enter_context(tc.tile_pool(name="dram", bufs=1, space="DRAM"))

        # ---------- weights first (wk on sync: needed soonest) -------------
        wk_sb = wpool.tile([P, ND, D], BF, name="wk", tag="w1", bufs=1)
        nc.sync.dma_start(wk_sb[:], ap_wqkvT[:, :, D:2 * D])
        wq_sb = wpool.tile([P, ND, D], BF, name="wq", tag="w3", bufs=1)
        nc.gpsimd.dma_start(wq_sb[:], ap_wqkvT[:, :, 0:D])
        wv_sb = wpool.tile([P, ND, D], BF, name="wv", tag="w2", bufs=1)
        nc.gpsimd.dma_start(wv_sb[:], ap_wqkvT[:, :, 2 * D:3 * D])

        # ---------- constants ----------
        qw_sb = const.tile([P, D], BF)      # q_ln_w broadcast to all partitions
        nc.gpsimd.dma_start(qw_sb[:], _bc_part(q_ln_w.ap()[None, :], P))
        kw_sb = const.tile([P, D], BF)
        nc.gpsimd.dma_start(kw_sb[:], _bc_part(k_ln_w.ap()[None, :], P))
        cos_sb = const.tile([P, NTQ, DH], BF)
        nc.gpsimd.dma_start(cos_sb[:], ap_cos)
        sin_sb = const.tile([P, NTQ, DH], BF)
        nc.gpsimd.dma_start(sin_sb[:], ap_sin)
        if c0_t is not None:
            c0_sb = const.tile([1, 3, D], BF)
            nc.gpsimd.dma_start(c0_sb[:], c0_t.ap()[None, :, :])
        if srow_t is not None:
            # colsums of W' per kind, broadcast to all partitions (the
            # raw-x projection's rank-1 mean correction)
            srow_sb = const.tile([P, 3, D], BF)
            nc.gpsimd.dma_start(srow_sb[:], _bc_part(srow_t.ap()[None], P))
        onesD_sb = const.tile([P, 1], BF)    # 1/D column for the stats matmul
        nc.vector.memset(onesD_sb[:], 1.0 / D)
        ones_row = const.tile([1, P], BF)
        nc.vector.memset(ones_row[:], 1.0)

        # ---------- long-lived tensors ----------
        v_sb = live.tile([P, NT, H, DH + 1], BF)
        nc.vector.memset(v_sb[:, :, :, DH:DH + 1], 1.0)
        krT = live.tile([P, ND, L], BF)
        qrT = live.tile([P, ND, LQ], BF)
        ctxT = live.tile([P, ND, LQ], BF)
        xsl = live.tile([P, ND, FD], BF)     # this quarter's x, e-major

        qr_d = dram.tile([LQ, D], BF, bufs=1)
        kq_d = dram.tile([LQ, D], BF, bufs=1)
        vq_d = dram.tile([LQ, D], BF, bufs=1)
        kg_d = dram.tile([L, D], BF, bufs=1)
        vg_d = dram.tile([L, D], BF, bufs=1)

        with tc.tile_pool(name="tmpA", bufs=1) as tmpA, \
             tc.tile_pool(name="ps1", bufs=1, space="PSUM") as ps1:

            # ---------- per-tile helpers ----------
            def project_tile(w_tile, jt, kind):
                """[128 tok, 1024] projection psum pair for local token tile
                jt. kind: 0=q, 1=k, 2=v (selects the folded-ln_b c0 row)."""
                pss = []
                for s2 in range(2):
                    ps = ps1.tile([P, FD], F32, name=f"pj{s2}",
                                  tag=f"pj{s2}", bufs=2)
                    for d in range(ND):
                        nc.tensor.matmul(ps[:],
                                         xsl[:, d, jt * P:(jt + 1) * P],
                                         w_tile[:, d, s2 * FD:(s2 + 1) * FD],
                                         start=(d == 0),
                                         stop=(c0_t is None and d == ND - 1))
                    if c0_t is not None:
                        nc.tensor.matmul(
                            ps[:], ones_row[0:1, :],
                            c0_sb[0:1, kind, s2 * FD:(s2 + 1) * FD],
                            start=False, stop=True)
                    pss.append(ps)
                return pss

            def evac_qk(pss, kind, jt, name):
                """PSUM -> raw. Fast path: raw' = mu*s - ps (the NEGATED
                pre-LN row; the per-token LN absorbs the sign via a negated
                scale pointer and is invariant to the missing 1/r)."""
                raw = tmpA.tile([P, D], BF, name=f"{name}raw", tag="raw",
                                bufs=3)
                if c0_t is None:
                    mu_col = rmu_sb[:, jt, 1:2]
                    with nc.allow_low_precision(reason="pre-LN row to bf16"):
                        for s2 in range(2):
                            sl = slice(s2 * FD, (s2 + 1) * FD)
                            nc.vector.scalar_tensor_tensor(
                                raw[:, sl], srow_sb[:, kind, sl], mu_col,
                                pss[s2][:], op0=OP.mult, op1=OP.subtract)
                else:
                    nc.scalar.copy(raw[:, 0:FD], pss[0][:])
                    nc.scalar.copy(raw[:, FD:D], pss[1][:])
                return raw

            def token_ln_rope(raw, w_row, scale, jt, dst_d, name):
                st6 = stat.tile([P, 2, 6], F32, name=f"{name}bs", tag="bs",
                                bufs=4)
                seg = raw[:].rearrange("p (s f) -> p s f", s=2)
                for s2 in range(2):
                    nc.vector.bn_stats(st6[:, s2, :], seg[:, s2, :])
                mv = stat.tile([P, 2], F32, name=f"{name}mv", tag="mv", bufs=4)
                nc.vector.bn_aggr(mv[:], st6[:])
                vep = stat.tile([P, 1], F32, name=f"{name}ve", tag="ve",
                                bufs=4)
                nc.vector.tensor_scalar(vep[:], mv[:, 1:2], 1.0, EPS,
                                        op0=OP.mult, op1=OP.add)
                r = stat.tile([P, 1], F32, name=f"{name}r", tag="lr", bufs=4)
                nc.scalar.activation(r[:], vep[:], AF.Sqrt)
                nc.vector.reciprocal(r[:], r[:])
                _rstd_refine(nc, stat, r, vep, (P, 1), "t")
                # fast path: raw is negated -> negate the LN scale (exact)
                rs = -scale if c0_t is None else scale
                if rs != 1.0:
                    nc.vector.tensor_scalar_mul(r[:], r[:], rs)
                nmb = stat.tile([P, 1], F32, name=f"{name}nmb", tag="nmb",
                                bufs=4)
                nc.vector.tensor_scalar(nmb[:], mv[:, 0:1], r[:], -1.0,
                                        op0=OP.mult, op1=OP.mult)
                nc.scalar.activation(raw[:], raw[:], AF.Identity,
                                     bias=nmb[:], scale=r[:])
                nc.vector.tensor_mul(raw[:], raw[:], w_row[:])
                xn = raw[:].rearrange("p (h j) -> p h j", j=DH)
                t2 = tmpA.tile([P, H, DH], BF, name=f"{name}t2", tag="rp2",
                               bufs=2)
                nc.vector.tensor_mul(t2[:, :, 0:DH // 2],
                                     xn[:, :, DH // 2:DH],
                                     _bc_heads(sin_sb[:, jt, 0:DH // 2], H))
                nc.vector.tensor_mul(t2[:, :, DH // 2:DH],
                                     xn[:, :, 0:DH // 2],
                                     _bc_heads(sin_sb[:, jt, DH // 2:DH], H))
                t3 = tmpA.tile([P, H, DH], BF, name=f"{name}t3", tag="rp3",
                               bufs=2)
                nc.gpsimd.tensor_mul(t3[:], xn,
                                     _bc_heads(cos_sb[:, jt, :], H))
                nc.gpsimd.tensor_add(t3[:], t3[:], t2[:])
                nc.sync.dma_start(dst_d[jt * P:(jt + 1) * P, :],
                                  t3[:].rearrange("p h j -> p (h j)"))

            # ---------- phase 1: stats + in-place LN (one local slab) ------
            nc.sync.dma_start(xsl[:], ap_xT)
            ps_s = ps1.tile([1, FD], F32, name="xs", tag="xs", bufs=1)
            ps_q = ps1.tile([1, FD], F32, name="xss", tag="xss", bufs=1)
            for d in range(ND):
                sq = tmpA.tile([P, FD], BF, name="xsq", tag="xsq", bufs=2)
                nc.scalar.activation(sq[:], xsl[:, d, :], AF.Square)
                nc.tensor.matmul(ps_s[:], onesD_sb[:], xsl[:, d, :],
                                 start=(d == 0), stop=(d == ND - 1))
                nc.tensor.matmul(ps_q[:], onesD_sb[:], sq[:],
                                 start=(d == 0), stop=(d == ND - 1))
            # ps_s = mean, ps_q = E[x^2]
            vep = stat.tile([1, FD], F32, name="xvep", tag="xvep")
            nc.scalar.activation(vep[:], ps_s[:], AF.Square)
            nc.vector.tensor_scalar(vep[:], vep[:], -1.0, EPS,
                                    op0=OP.mult, op1=OP.add)
            nc.vector.scalar_tensor_tensor(vep[:], ps_q[:], 1.0, vep[:],
                                           op0=OP.mult, op1=OP.add)
            r = stat.tile([1, FD], F32, name="xr", tag="xr")
            nc.scalar.activation(r[:], vep[:], AF.Sqrt)
            nc.vector.reciprocal(r[:], r[:])
            _rstd_refine(nc, stat, r, vep, (1, FD), "x")
            if c0_t is None:
                # token-major [-r | mu] columns via tiny K=1 matmuls; x
                # stays RAW (projections start without waiting for stats)
                rows = stat.tile([1, 2, FD], BF, name="xrows", tag="xrows",
                                 bufs=1)
                with nc.allow_low_precision(reason="stat rows to bf16"):
                    nc.vector.tensor_scalar_mul(rows[:, 0, :], r[:], -1.0)
                    nc.vector.tensor_copy(rows[:, 1, :], ps_s[:])
                rmu_ps = ps1.tile([P, NTQ, 2], F32, name="rmu", tag="rmu",
                                  bufs=1)
                for jt in range(NTQ):
                    for q2 in range(2):
                        nc.tensor.matmul(
                            rmu_ps[:, jt, q2:q2 + 1],
                            rows[0:1, q2, jt * P:(jt + 1) * P],
                            ones_row[0:1, 0:1], start=True, stop=True)
                rmu_sb = stat.tile([P, NTQ, 2], F32, name="rmusb",
                                   tag="rmusb", bufs=1)
                nc.vector.tensor_copy(rmu_sb[:], rmu_ps[:])
            else:
                rows = stat.tile([1, 2, FD], BF, name="xrows", tag="xrows",
                                 bufs=2)
                nc.vector.tensor_copy(rows[:, 0, :], r[:])
                with nc.allow_low_precision(reason="mu*r row to bf16"):
                    nc.vector.tensor_mul(rows[:, 1, :], ps_s[:], r[:])
                bc_ps = ps1.tile([P, 2, FD], F32, name="bc", tag="bc",
                                 bufs=1)
                nc.tensor.matmul(bc_ps[:, 0, :], ones_row[0:1, :],
                                 rows[:, 0, :], start=True, stop=True)
                nc.tensor.matmul(bc_ps[:, 1, :], ones_row[0:1, :],
                                 rows[:, 1, :], start=True, stop=True)
                rbmr = tmpA.tile([P, 2, FD], BF, name="rbmr", tag="rbmr",
                                 bufs=1)
                nc.scalar.copy(rbmr[:], bc_ps[:])
                # in-place: x <- x*r - mu*r  (e-major)
                for d in range(ND):
                    nc.vector.tensor_mul(xsl[:, d, :], xsl[:, d, :],
                                         rbmr[:, 0, :])
                    nc.gpsimd.tensor_sub(xsl[:, d, :], xsl[:, d, :],
                                         rbmr[:, 1, :])

            # ---------- local k quarter -> AllGather (gpsimd) ----------
            for jt in range(NTQ):
                pss = project_tile(wk_sb, jt, 1)
                raw = evac_qk(pss, 1, jt, "k")
                token_ln_rope(raw, kw_sb, 1.0, jt, kq_d, "k")
            # NOTE: the sim cost model charges collectives ~120us to the
            # issuing engine queue; real HW is a doorbell + ~15us (4-rank
            # 1MB AllGather). Engines are picked so nothing
            # latency-critical sits behind a collective in its queue.
            nc.gpsimd.collective_compute(
                "AllGather", OP.bypass, replica_groups=GROUPS,
                ins=[kq_d[:].opt()], outs=[kg_d[:].opt()])

            # ---------- local q quarter -> qrT (transpose early: it gates
            # the first scores together with krT slab 0) ----------
            for jt in range(NTQ):
                pss = project_tile(wq_sb, jt, 0)
                raw = evac_qk(pss, 0, jt, "q")
                token_ln_rope(raw, qw_sb, DH ** -0.5, jt, qr_d, "q")
            nc.sync.dma_start_transpose(qrT[:], qr_d[:])

            # ---------- local v quarter ----------
            for jt in range(NTQ):
                pss = project_tile(wv_sb, jt, 2)
                vq_sb = tmpA.tile([P, D], BF, name="vq", tag="vq", bufs=2)
                if c0_t is None:
                    # v = (ps - mu*s_v)*r = (mu*s_v - ps) * (-r)
                    with nc.allow_low_precision(reason="v to bf16"):
                        for s2 in range(2):
                            sl = slice(s2 * FD, (s2 + 1) * FD)
                            vtmp = tmpA.tile([P, FD], BF, name="vtmp",
                                             tag="vtmp", bufs=2)
                            nc.vector.scalar_tensor_tensor(
                                vtmp[:], srow_sb[:, 2, sl],
                                rmu_sb[:, jt, 1:2], pss[s2][:],
                                op0=OP.mult, op1=OP.subtract)
                            nc.vector.tensor_scalar_mul(
                                vq_sb[:, sl], vtmp[:], rmu_sb[:, jt, 0:1])
                else:
                    nc.vector.tensor_copy(vq_sb[:, 0:FD], pss[0][:])
                    nc.vector.tensor_copy(vq_sb[:, FD:D], pss[1][:])
                nc.sync.dma_start(vq_d[jt * P:(jt + 1) * P, :], vq_sb[:])
            nc.gpsimd.collective_compute(
                "AllGather", OP.bypass, replica_groups=GROUPS,
                ins=[vq_d[:].opt()], outs=[vg_d[:].opt()])
            for j in range(NSL):
                nc.sync.dma_start_transpose(
                    krT[:, :, j * FD:(j + 1) * FD],
                    kg_d[j * FD:(j + 1) * FD, :])
            for st in range(NT):
                src = vg_d[st * P:(st + 1) * P, :].rearrange(
                    "p (h e) -> p h e", e=DH)
                nc.sync.dma_start(v_sb[:, st, :, 0:DH], src)

        # wq slot done; load w_out for the final projection (SWDGE).
        wo_sb = wpool.tile([P, ND, D], BF, name="wo", tag="w3", bufs=1)
        nc.gpsimd.dma_start(wo_sb[:], ap_woutT)

        with tc.tile_pool(name="tmpC", bufs=1) as tmpC, \
             tc.tile_pool(name="ps2", bufs=1, space="PSUM") as ps2:
            # Wave-1 of the output projection (contraction d=0..3) runs
            # right after et=3 so its matmuls hide under the ACT-bound
            # attention window; wave-2 accumulates on top after et=7.
            osb1 = [[None] * 2 for _ in range(NTQ)]

            def out_wave(wave):
                ds = range(0, ND // 2) if wave == 0 else range(ND // 2, ND)
                for tt in range(NTQ):
                    o_sb = (None if wave == 0 else
                            tmpC.tile([P, D], F32, name="osb", tag="osb",
                                      bufs=2))
                    for s2 in range(2):
                        ps = ps2.tile([P, FD], F32, name="ops", tag="rbps",
                                      bufs=2)
                        for d in ds:
                            nc.tensor.matmul(
                                ps[:], ctxT[:, d, tt * P:(tt + 1) * P],
                                wo_sb[:, d, s2 * FD:(s2 + 1) * FD],
                                start=(d == ds[0]), stop=(d == ds[-1]))
                        if wave == 0:
                            half = tmpC.tile([P, FD], F32,
                                             name=f"o1_{tt}_{s2}",
                                             tag=f"o1_{tt}_{s2}", bufs=1)
                            osb1[tt][s2] = half
                            nc.vector.tensor_copy(half[:], ps[:])
                        else:
                            nc.vector.scalar_tensor_tensor(
                                o_sb[:, s2 * FD:(s2 + 1) * FD], ps[:], 1.0,
                                osb1[tt][s2][:], op0=OP.mult, op1=OP.add)
                    if wave == 1:
                        nc.sync.dma_start(out.ap()[tt * P:(tt + 1) * P, :],
                                          o_sb[:])

            # ---------- attention (head pairs, chunked exp) -------
            for et in range(ND):
                hA, hB = 2 * et, 2 * et + 1
                ctx_a = ps2.tile([DH + 1, LQ], F32, name="ctxa", tag="ctx",
                                 bufs=2)
                ctx_b = ps2.tile([DH + 1, LQ], F32, name="ctxb", tag="ctx",
                                 bufs=2)
                kA = krT[0:DH, et, :]
                kB = krT[DH:P, et, :]
                qA = qrT[0:DH, et, :]
                qB = qrT[DH:P, et, :]
                for g in range(NT // 2):
                    st0, st1 = 2 * g, 2 * g + 1
                    spsA = ps2.tile([P, 2, LQ], F32, name="spsA",
                                    tag="sps", bufs=2)
                    spsB = ps2.tile([P, 2, LQ], F32, name="spsB",
                                    tag="sps", bufs=2)
                    nc.tensor.matmul(spsA[:, 0, :],
                                     kA[:, st0 * P:(st0 + 1) * P], qA,
                                     start=True, stop=True)
                    nc.tensor.matmul(spsB[:, 0, :],
                                     kB[:, st0 * P:(st0 + 1) * P], qB,
                                     start=True, stop=True)
                    nc.tensor.matmul(spsA[:, 1, :],
                                     kA[:, st1 * P:(st1 + 1) * P], qA,
                                     start=True, stop=True)
                    nc.tensor.matmul(spsB[:, 1, :],
                                     kB[:, st1 * P:(st1 + 1) * P], qB,
                                     start=True, stop=True)
                    expA = tmpC.tile([P, 2, LQ], BF, name="expA",
                                     tag="exp", bufs=5)
                    expB = tmpC.tile([P, 2, LQ], BF, name="expB",
                                     tag="exp", bufs=5)
                    nc.scalar.activation(expA[:], spsA[:], AF.Exp)
                    nc.scalar.activation(expB[:], spsB[:], AF.Exp)
                    for j, st in ((0, st0), (1, st1)):
                        nc.tensor.matmul(ctx_a[:], v_sb[:, st, hA, :],
                                         expA[:, j, :],
                                         start=(st == 0),
                                         stop=(st == NT - 1))
                        nc.tensor.matmul(ctx_b[:], v_sb[:, st, hB, :],
                                         expB[:, j, :],
                                         start=(st == 0),
                                         stop=(st == NT - 1))
                for hh, cps in ((hA, ctx_a), (hB, ctx_b)):
                    half = (hh % 2) * DH
                    rrow = stat.tile([1, LQ], BF, name="rrow", tag="rrow",
                                     bufs=2)
                    with nc.allow_low_precision(reason="softmax denom"):
                        nc.vector.reciprocal(rrow[:], cps[DH:DH + 1, :])
                    rb_ps = ps2.tile([DH, LQ], F32, name="rbps", tag="rbps",
                                     bufs=2)
                    nc.tensor.matmul(rb_ps[:], ones_row[0:1, 0:DH], rrow[:],
                                     start=True, stop=True)
                    rb = tmpC.tile([DH, LQ], BF, name="rb", tag="rb",
                                   bufs=2)
                    nc.vector.tensor_copy(rb[:], rb_ps[:])
                    nc.vector.tensor_mul(ctxT[half:half + DH, et, :],
                                         cps[0:DH, :], rb[:])
                if et == ND // 2 - 1:
                    # low priority: PE picks these up in exp-wait gaps
                    with tc.high_priority(offset=-1000000):
                        out_wave(0)

            # ---------- output projection, wave 2 ----------
            out_wave(1)


_NC_CACHE = {}


def build_nc(do_compile=True, with_c0=False):
    nc = bacc.Bacc("TRN2", target_bir_lowering=False, debug=False,
                   num_devices=NCORES)
    _emit(nc, with_c0)
    if do_compile:
        nc.compile()
    return nc


def _get_nc(with_c0=False):
    if with_c0 not in _NC_CACHE:
        _NC_CACHE[with_c0] = build_nc(do_compile=True, with_c0=with_c0)
    return _NC_CACHE[with_c0]


def _build_tables():
    inv_freq = 1.0 / (ROPE_BASE ** (np.arange(0, DH, 2, dtype=np.float32) / DH))
    t = np.arange(L, dtype=np.float32)
    freqs = np.outer(t, inv_freq)                       # [L, 32]
    cos = np.concatenate([np.cos(freqs)] * 2, axis=1)   # [L, 64]
    sin = np.concatenate([np.sin(freqs)] * 2, axis=1)
    sign = np.where(np.arange(DH) < DH // 2, -1.0, 1.0).astype(np.float32)
    return (cos.astype(ml_dtypes.bfloat16),
            (sin * sign[None, :]).astype(ml_dtypes.bfloat16))


def make_in_maps(x, ln_w, ln_b, w_qkv, q_ln_w, k_ln_w, w_out):
    w_qkv = np.asarray(w_qkv, np.float32)
    ln_w = np.asarray(ln_w, np.float32)
    ln_b = np.asarray(ln_b, np.float32)
    # fold the x-layernorm affine into the projection (exact):
    #   qkv = ((x-mu)*r * ln_w + ln_b) @ W^T
    #       = ((x-mu)*r) @ (W*ln_w)^T + (W @ ln_b)
    wf = w_qkv * ln_w[None, :]
    c0 = (w_qkv @ ln_b).reshape(3, D)
    with_c0 = bool(np.any(c0 != 0.0))
    wqkvT = np.ascontiguousarray(wf.T).astype(ml_dtypes.bfloat16)
    woutT = np.ascontiguousarray(np.asarray(w_out, np.float32).T).astype(
        ml_dtypes.bfloat16)
    cos_t, sin_t = _build_tables()
    x = np.asarray(x, np.float32)
    in_maps = []
    for c in range(NCORES):
        b, s = c // 4, c % 4
        xq = x[b, s * LQ:(s + 1) * LQ]
        xT = np.ascontiguousarray(xq.T).astype(ml_dtypes.bfloat16)
        im = {
            "xT": xT, "wqkvT": wqkvT, "woutT": woutT,
            "q_ln_w": np.asarray(q_ln_w, np.float32).astype(ml_dtypes.bfloat16),
            "k_ln_w": np.asarray(k_ln_w, np.float32).astype(ml_dtypes.bfloat16),
            "cos_t": np.ascontiguousarray(cos_t[s * LQ:(s + 1) * LQ]),
            "sin_t": np.ascontiguousarray(sin_t[s * LQ:(s + 1) * LQ]),
        }
        if with_c0:
            im["c0_t"] = c0.astype(ml_dtypes.bfloat16)
        else:
            im["srow_t"] = wf.sum(axis=1).reshape(3, D).astype(
                ml_dtypes.bfloat16)
        in_maps.append(im)
    return in_maps, with_c0


def kernel(x, ln_w, ln_b, w_qkv, q_ln_w, k_ln_w, w_out, **run_kwargs):
    in_maps, with_c0 = make_in_maps(x, ln_w, ln_b, w_qkv, q_ln_w, k_ln_w,
                                    w_out)
    nc = _get_nc(with_c0)
    res = run_bass_kernel_spmd(nc, in_maps, core_ids=list(range(NCORES)),
                               **run_kwargs)
    out = np.zeros((B, L, D), np.float32)
    for c in range(NCORES):
        b, s = c // 4, c % 4
        out[b, s * LQ:(s + 1) * LQ, :] = res.results[c]["out"]
    return out


# revision 39
# speedup vs baseline: 1.1360x; 1.1360x over previous
"""Trainium2 Bass kernel: fused MHA block (LN -> QKV -> q/k per-token LN ->
RoPE -> SDPA -> out-proj), SPMD over 8 NeuronCores.

Sharding (v3): core c handles batch b = c//4 and token quarter s = c%4 in
GLOBAL token order. Each core projects q/k/v only for its own 512-token
quarter (no cross-core redundancy), then the 4 cores of a batch AllGather
the roped keys and the values; every core runs attention for its 512
queries over all 2048 keys. Host concatenates 8 [512, 1024] output slices.

Design notes (all matmuls bf16 with f32 PSUM accumulation):
  - ln_w folded into w_qkv on the host (W' = W * ln_w); nonzero ln_b enters
    as one K=1 ones-matmul accumulate (c0 = W @ ln_b) per projection half.
  - x normalized IN PLACE in the e-major x slab tile: stats via ones(1/D)
    matmuls; r and mu*r rows broadcast across partitions with K=1 matmuls
    into PSUM (no DRAM bounce).
  - per-token q/k LN: bn_stats on DVE, affine applied on ACT via per-token
    scale/bias pointers (in place); RoPE sin-mul on DVE, cos-mul + add on
    Pool.
  - AllGather #1: roped k quarter [512, D] -> [2048, D], then 4 XBAR
    transposes into feature-major krT. AllGather #2: v quarter -> v_sb
    (strided loads add the fused softmax-denominator ones column).
  - attention: scoresT = krT.T @ qrT per head pair, exp on ACT (the binding
    engine); AV ones-column produces the softmax denominator; denominator
    reciprocal broadcast via K=1 matmul. PSUM evacuations on DVE so ACT
    does nothing but exp.
  - transposes (XBAR) from SP, weight DMAs split gpsimd/sync.
"""

import numpy as np
import ml_dtypes

import concourse.bass as bass
import concourse.mybir as mybir
import concourse.tile as tile
from concourse import bacc
from concourse.bass_utils import run_bass_kernel_spmd

B, L, D, H, DH = 2, 2048, 1024, 16, 64
EPS = 1e-5
ROPE_BASE = 10000.0
NCORES = 8
LQ = L // 4
P = 128
ND = D // P      # 8 feature tiles of 128
NT = L // P      # 16 key token tiles
NTQ = LQ // P    # 4 local token tiles
FD = 512         # psum bank free size (f32)
NSL = L // FD    # 4 key slabs of 512
BF = mybir.dt.bfloat16
F32 = mybir.dt.float32
AF = mybir.ActivationFunctionType
OP = mybir.AluOpType
GROUPS = [[0, 1, 2, 3], [4, 5, 6, 7]]


def _bc_part(ap, parts):
    """Partition-broadcast (step 0) of a [1, ...] DRAM AP to `parts` rows."""
    return bass.AP(tensor=ap.tensor, offset=ap.offset,
                   ap=[[0, parts]] + list(ap.ap[1:]))


def _bc_heads(ap2, n, at=1):
    """Insert a step-0 dim of size n at free position `at` of a 2D sbuf AP."""
    dims = list(ap2.ap)
    return bass.AP(tensor=ap2.tensor, offset=ap2.offset,
                   ap=dims[:at] + [[0, n]] + dims[at:])


def _emit(nc, with_c0):
    xT = nc.dram_tensor("xT", [D, LQ], BF, kind="ExternalInput")
    wqkvT = nc.dram_tensor("wqkvT", [D, 3 * D], BF, kind="ExternalInput")
    woutT = nc.dram_tensor("woutT", [D, D], BF, kind="ExternalInput")
    c0_t = (nc.dram_tensor("c0_t", [3, D], BF, kind="ExternalInput")
            if with_c0 else None)
    srow_t = (None if with_c0 else
              nc.dram_tensor("srow_t", [3, D], BF, kind="ExternalInput"))
    q_ln_w = nc.dram_tensor("q_ln_w", [D], BF, kind="ExternalInput")
    k_ln_w = nc.dram_tensor("k_ln_w", [D], BF, kind="ExternalInput")
    cos_t = nc.dram_tensor("cos_t", [LQ, DH], BF, kind="ExternalInput")
    sin_t = nc.dram_tensor("sin_t", [LQ, DH], BF, kind="ExternalInput")
    out = nc.dram_tensor("out", [LQ, D], F32, kind="ExternalOutput")

    with tile.TileContext(nc) as tc:
        _body(nc, tc, xT, wqkvT, woutT, c0_t, srow_t, q_ln_w, k_ln_w,
              cos_t, sin_t, out)
    return nc


def _rstd_refine(nc, pool, r, vareps, shape, name):
    """One Newton step for r ~= rsqrt(varep): r' = r*(1.5 - 0.5*varep*r^2).
    Guards against ACT sqrt LUT error on hardware. In-place on r."""
    t = pool.tile(list(shape), F32, name=f"{name}_nt", tag=f"{name}_nt", bufs=2)
    nc.scalar.activation(t[:], r[:], AF.Square)
    nc.vector.tensor_mul(t[:], t[:], vareps[:])
    nc.vector.tensor_scalar(t[:], t[:], -0.5, 1.5, op0=OP.mult, op1=OP.add)
    nc.vector.tensor_mul(r[:], r[:], t[:])


def _body(nc, tc, xT, wqkvT, woutT, c0_t, srow_t, q_ln_w, k_ln_w,
          cos_t, sin_t, out):
    import contextlib
    ap_xT = xT.ap().rearrange("(nd p) t -> p nd t", p=P)
    ap_wqkvT = wqkvT.ap().rearrange("(nd p) e -> p nd e", p=P)
    ap_woutT = woutT.ap().rearrange("(nd p) e -> p nd e", p=P)
    ap_cos = cos_t.ap().rearrange("(tt p) j -> p tt j", p=P)
    ap_sin = sin_t.ap().rearrange("(tt p) j -> p tt j", p=P)

    ctx = contextlib.ExitStack()
    with ctx:
        const = ctx.enter_context(tc.tile_pool(name="const", bufs=1))
        wpool = ctx.enter_context(tc.tile_pool(name="wp", bufs=1))
        live = ctx.enter_context(tc.tile_pool(name="live", bufs=1))
        stat = ctx.enter_context(tc.tile_pool(name="stat", bufs=1))
        dram = ctx.enter_context(tc.tile_pool(name="dram", bufs=1, space="DRAM"))

        # ---------- weights first (wk on sync: needed soonest) -------------
        wk_sb = wpool.tile([P, ND, D], BF, name="wk", tag="w1", bufs=1)
        nc.sync.dma_start(wk_sb[:], ap_wqkvT[:, :, D:2 * D])
        wq_sb = wpool.tile([P, ND, D], BF, name="wq", tag="w3", bufs=1)
        nc.gpsimd.dma_start(wq_sb[:], ap_wqkvT[:, :, 0:D])
        wv_sb = wpool.tile([P, ND, D], BF, name="wv", tag="w2", bufs=1)
        nc.gpsimd.dma_start(wv_sb[:], ap_wqkvT[:, :, 2 * D:3 * D])

        # ---------- constants ----------
        qw_sb = const.tile([P, D], BF)      # q_ln_w broadcast to all partitions
        nc.gpsimd.dma_start(qw_sb[:], _bc_part(q_ln_w.ap()[None, :], P))
        kw_sb = const.tile([P, D], BF)
        nc.gpsimd.dma_start(kw_sb[:], _bc_part(k_ln_w.ap()[None, :], P))
        cos_sb = const.tile([P, NTQ, DH], BF)
        nc.gpsimd.dma_start(cos_sb[:], ap_cos)
        sin_sb = const.tile([P, NTQ, DH], BF)
        nc.gpsimd.dma_start(sin_sb[:], ap_sin)
        if c0_t is not None:
            c0_sb = const.tile([1, 3, D], BF)
            nc.gpsimd.dma_start(c0_sb[:], c0_t.ap()[None, :, :])
        if srow_t is not None:
            # colsums of W' per kind, broadcast to all partitions (the
            # raw-x projection's rank-1 mean correction)
            srow_sb = const.tile([P, 3, D], BF)
            nc.gpsimd.dma_start(srow_sb[:], _bc_part(srow_t.ap()[None], P))
        onesD_sb = const.tile([P, 1], BF)    # 1/D column for the stats matmul
        nc.vector.memset(onesD_sb[:], 1.0 / D)
        ones_row = const.tile([1, P], BF)
        nc.vector.memset(ones_row[:], 1.0)

        # ---------- long-lived tensors ----------
        v_sb = live.tile([P, NT, H, DH + 1], BF)
        nc.vector.memset(v_sb[:, :, :, DH:DH + 1], 1.0)
        krT = live.tile([P, ND, L], BF)
        qrT = live.tile([P, ND, LQ], BF)
        ctxT = live.tile([P, ND, LQ], BF)
        xsl = live.tile([P, ND, FD], BF)     # this quarter's x, e-major

        qr_d = dram.tile([LQ, D], BF, bufs=1)
        kq_d = dram.tile([LQ, D], BF, bufs=1)
        vq_d = dram.tile([LQ, D], BF, bufs=1)
        kg_d = dram.tile([L, D], BF, bufs=1)
        vg_d = dram.tile([L, D], BF, bufs=1)

        with tc.tile_pool(name="tmpA", bufs=1) as tmpA, \
             tc.tile_pool(name="ps1", bufs=1, space="PSUM") as ps1:

            # ---------- per-tile helpers ----------
            def project_tile(w_tile, jt, kind):
                """[128 tok, 1024] projection psum pair for local token tile
                jt. kind: 0=q, 1=k, 2=v (selects the folded-ln_b c0 row)."""
                pss = []
                for s2 in range(2):
                    ps = ps1.tile([P, FD], F32, name=f"pj{s2}",
                                  tag=f"pj{s2}", bufs=2)
                    for d in range(ND):
                        nc.tensor.matmul(ps[:],
                                         xsl[:, d, jt * P:(jt + 1) * P],
                                         w_tile[:, d, s2 * FD:(s2 + 1) * FD],
                                         start=(d == 0),
                                         stop=(c0_t is None and d == ND - 1))
                    if c0_t is not None:
                        nc.tensor.matmul(
                            ps[:], ones_row[0:1, :],
                            c0_sb[0:1, kind, s2 * FD:(s2 + 1) * FD],
                            start=False, stop=True)
                    pss.append(ps)
                return pss

            def evac_qk(pss, kind, jt, name):
                """PSUM -> raw. Fast path: raw' = mu*s - ps (the NEGATED
                pre-LN row; the per-token LN absorbs the sign via a negated
                scale pointer and is invariant to the missing 1/r)."""
                raw = tmpA.tile([P, D], BF, name=f"{name}raw", tag="raw",
                                bufs=3)
                if c0_t is None:
                    mu_col = rmu_sb[:, jt, 1:2]
                    with nc.allow_low_precision(reason="pre-LN row to bf16"):
                        for s2 in range(2):
                            sl = slice(s2 * FD, (s2 + 1) * FD)
                            nc.vector.scalar_tensor_tensor(
                                raw[:, sl], srow_sb[:, kind, sl], mu_col,
                                pss[s2][:], op0=OP.mult, op1=OP.subtract)
                else:
                    nc.scalar.copy(raw[:, 0:FD], pss[0][:])
                    nc.scalar.copy(raw[:, FD:D], pss[1][:])
                return raw

            def token_ln_rope(raw, w_row, scale, jt, dst_d, name):
                st6 = stat.tile([P, 2, 6], F32, name=f"{name}bs", tag="bs",
                                bufs=4)
                seg = raw[:].rearrange("p (s f) -> p s f", s=2)
                for s2 in range(2):
                    nc.vector.bn_stats(st6[:, s2, :], seg[:, s2, :])
                mv = stat.tile([P, 2], F32, name=f"{name}mv", tag="mv", bufs=4)
                nc.vector.bn_aggr(mv[:], st6[:])
                vep = stat.tile([P, 1], F32, name=f"{name}ve", tag="ve",
                                bufs=4)
                nc.vector.tensor_scalar(vep[:], mv[:, 1:2], 1.0, EPS,
                                        op0=OP.mult, op1=OP.add)
                r = stat.tile([P, 1], F32, name=f"{name}r", tag="lr", bufs=4)
                nc.scalar.activation(r[:], vep[:], AF.Sqrt)
                nc.vector.reciprocal(r[:], r[:])
                _rstd_refine(nc, stat, r, vep, (P, 1), "t")
                # fast path: raw is negated -> negate the LN scale (exact)
                rs = -scale if c0_t is None else scale
                if rs != 1.0:
                    nc.vector.tensor_scalar_mul(r[:], r[:], rs)
                nmb = stat.tile([P, 1], F32, name=f"{name}nmb", tag="nmb",
                                bufs=4)
                nc.vector.tensor_scalar(nmb[:], mv[:, 0:1], r[:], -1.0,
                                        op0=OP.mult, op1=OP.mult)
                nc.scalar.activation(raw[:], raw[:], AF.Identity,
                                     bias=nmb[:], scale=r[:])
                nc.vector.tensor_mul(raw[:], raw[:], w_row[:])
                xn = raw[:].rearrange("p (h j) -> p h j", j=DH)
                t2 = tmpA.tile([P, H, DH], BF, name=f"{name}t2", tag="rp2",
                               bufs=2)
                nc.vector.tensor_mul(t2[:, :, 0:DH // 2],
                                     xn[:, :, DH // 2:DH],
                                     _bc_heads(sin_sb[:, jt, 0:DH // 2], H))
                nc.vector.tensor_mul(t2[:, :, DH // 2:DH],
                                     xn[:, :, 0:DH // 2],
                                     _bc_heads(sin_sb[:, jt, DH // 2:DH], H))
                t3 = tmpA.tile([P, H, DH], BF, name=f"{name}t3", tag="rp3",
                               bufs=2)
                nc.gpsimd.tensor_mul(t3[:], xn,
                                     _bc_heads(cos_sb[:, jt, :], H))
                nc.gpsimd.tensor_add(t3[:], t3[:], t2[:])
                nc.sync.dma_start(dst_d[jt * P:(jt + 1) * P, :],
                                  t3[:].rearrange("p h j -> p (h j)"))

            # ---------- phase 1: stats + in-place LN (one local slab) ------
            nc.sync.dma_start(xsl[:], ap_xT)
            ps_s = ps1.tile([1, FD], F32, name="xs", tag="xs", bufs=1)
            ps_q = ps1.tile([1, FD], F32, name="xss", tag="xss", bufs=1)
            for d in range(ND):
                sq = tmpA.tile([P, FD], BF, name="xsq", tag="xsq", bufs=2)
                nc.scalar.activation(sq[:], xsl[:, d, :], AF.Square)
                nc.tensor.matmul(ps_s[:], onesD_sb[:], xsl[:, d, :],
                                 start=(d == 0), stop=(d == ND - 1))
                nc.tensor.matmul(ps_q[:], onesD_sb[:], sq[:],
                                 start=(d == 0), stop=(d == ND - 1))
            # ps_s = mean, ps_q = E[x^2]
            vep = stat.tile([1, FD], F32, name="xvep", tag="xvep")
            nc.scalar.activation(vep[:], ps_s[:], AF.Square)
            nc.vector.tensor_scalar(vep[:], vep[:], -1.0, EPS,
                                    op0=OP.mult, op1=OP.add)
            nc.vector.scalar_tensor_tensor(vep[:], ps_q[:], 1.0, vep[:],
                                           op0=OP.mult, op1=OP.add)
            r = stat.tile([1, FD], F32, name="xr", tag="xr")
            nc.scalar.activation(r[:], vep[:], AF.Sqrt)
            nc.vector.reciprocal(r[:], r[:])
            _rstd_refine(nc, stat, r, vep, (1, FD), "x")
            if c0_t is None:
                # token-major [-r | mu] columns via tiny K=1 matmuls; x
                # stays RAW (projections start without waiting for stats)
                rows = stat.tile([1, 2, FD], BF, name="xrows", tag="xrows",
                                 bufs=1)
                with nc.allow_low_precision(reason="stat rows to bf16"):
                    nc.vector.tensor_scalar_mul(rows[:, 0, :], r[:], -1.0)
                    nc.vector.tensor_copy(rows[:, 1, :], ps_s[:])
                rmu_ps = ps1.tile([P, NTQ, 2], F32, name="rmu", tag="rmu",
                                  bufs=1)
                for jt in range(NTQ):
                    for q2 in range(2):
                        nc.tensor.matmul(
                            rmu_ps[:, jt, q2:q2 + 1],
                            rows[0:1, q2, jt * P:(jt + 1) * P],
                            ones_row[0:1, 0:1], start=True, stop=True)
                rmu_sb = stat.tile([P, NTQ, 2], F32, name="rmusb",
                                   tag="rmusb", bufs=1)
                nc.vector.tensor_copy(rmu_sb[:], rmu_ps[:])
            else:
                rows = stat.tile([1, 2, FD], BF, name="xrows", tag="xrows",
                                 bufs=2)
                nc.vector.tensor_copy(rows[:, 0, :], r[:])
                with nc.allow_low_precision(reason="mu*r row to bf16"):
                    nc.vector.tensor_mul(rows[:, 1, :], ps_s[:], r[:])
                bc_ps = ps1.tile([P, 2, FD], F32, name="bc", tag="bc",
                                 bufs=1)
                nc.tensor.matmul(bc_ps[:, 0, :], ones_row[0:1, :],
                                 rows[:, 0, :], start=True, stop=True)
                nc.tensor.matmul(bc_ps[:, 1, :], ones_row[0:1, :],
                                 rows[:, 1, :], start=True, stop=True)
                rbmr = tmpA.tile([P, 2, FD], BF, name="rbmr", tag="rbmr",
                                 bufs=1)
                nc.scalar.copy(rbmr[:], bc_ps[:])
                # in-place: x <- x*r - mu*r  (e-major)
                for d in range(ND):
                    nc.vector.tensor_mul(xsl[:, d, :], xsl[:, d, :],
                                         rbmr[:, 0, :])
                    nc.gpsimd.tensor_sub(xsl[:, d, :], xsl[:, d, :],
                                         rbmr[:, 1, :])

            # ---------- local k quarter -> AllGather (gpsimd) ----------
            for jt in range(NTQ):
                pss = project_tile(wk_sb, jt, 1)
                raw = evac_qk(pss, 1, jt, "k")
                token_ln_rope(raw, kw_sb, 1.0, jt, kq_d, "k")
            # NOTE: the sim cost model charges collectives ~120us to the
            # issuing engine queue; real HW is a doorbell + ~15us (4-rank
            # 1MB AllGather). Engines are picked so nothing
            # latency-critical sits behind a collective in its queue.
            nc.gpsimd.collective_compute(
                "AllGather", OP.bypass, replica_groups=GROUPS,
                ins=[kq_d[:].opt()], outs=[kg_d[:].opt()])

            # ---------- local q quarter -> qrT (transpose early: it gates
            # the first scores together with krT slab 0) ----------
            for jt in range(NTQ):
                pss = project_tile(wq_sb, jt, 0)
                raw = evac_qk(pss, 0, jt, "q")
                token_ln_rope(raw, qw_sb, DH ** -0.5, jt, qr_d, "q")
            nc.sync.dma_start_transpose(qrT[:], qr_d[:])

            # ---------- local v quarter ----------
            for jt in range(NTQ):
                pss = project_tile(wv_sb, jt, 2)
                vq_sb = tmpA.tile([P, D], BF, name="vq", tag="vq", bufs=2)
                if c0_t is None:
                    # v = (ps - mu*s_v)*r = (mu*s_v - ps) * (-r)
                    with nc.allow_low_precision(reason="v to bf16"):
                        for s2 in range(2):
                            sl = slice(s2 * FD, (s2 + 1) * FD)
                            vtmp = tmpA.tile([P, FD], BF, name="vtmp",
                                             tag="vtmp", bufs=2)
                            nc.vector.scalar_tensor_tensor(
                                vtmp[:], srow_sb[:, 2, sl],
                                rmu_sb[:, jt, 1:2], pss[s2][:],
                                op0=OP.mult, op1=OP.subtract)
                            nc.vector.tensor_scalar_mul(
                                vq_sb[:, sl], vtmp[:], rmu_sb[:, jt, 0:1])
                else:
                    nc.vector.tensor_copy(vq_sb[:, 0:FD], pss[0][:])
                    nc.vector.tensor_copy(vq_sb[:, FD:D], pss[1][:])
                nc.sync.dma_start(vq_d[jt * P:(jt + 1) * P, :], vq_sb[:])
            nc.gpsimd.collective_compute(
                "AllGather", OP.bypass, replica_groups=GROUPS,
                ins=[vq_d[:].opt()], outs=[vg_d[:].opt()])
            for j in range(NSL):
                nc.sync.dma_start_transpose(
                    krT[:, :, j * FD:(j + 1) * FD],
                    kg_d[j * FD:(j + 1) * FD, :])
            for st in range(NT):
                src = vg_d[st * P:(st + 1) * P, :].rearrange(
                    "p (h e) -> p h e", e=DH)
                nc.sync.dma_start(v_sb[:, st, :, 0:DH], src)

        # wq slot done; load w_out for the final projection (SWDGE).
        wo_sb = wpool.tile([P, ND, D], BF, name="wo", tag="w3", bufs=1)
        nc.gpsimd.dma_start(wo_sb[:], ap_woutT)

        with tc.tile_pool(name="tmpC", bufs=1) as tmpC, \
             tc.tile_pool(name="ps2", bufs=1, space="PSUM") as ps2:
            # Wave-1 of the output projection (contraction d=0..3) runs
            # right after et=3 so its matmuls hide under the ACT-bound
            # attention window; wave-2 accumulates on top after et=7.
            osb1 = [[None] * 2 for _ in range(NTQ)]

            def out_wave(wave):
                ds = range(0, ND // 2) if wave == 0 else range(ND // 2, ND)
                for tt in range(NTQ):
                    o_sb = (None if wave == 0 else
                            tmpC.tile([P, D], F32, name="osb", tag="osb",
                                      bufs=2))
                    for s2 in range(2):
                        ps = ps2.tile([P, FD], F32, name="ops", tag="rbps",
                                      bufs=2)
                        for d in ds:
                            nc.tensor.matmul(
                                ps[:], ctxT[:, d, tt * P:(tt + 1) * P],
                                wo_sb[:, d, s2 * FD:(s2 + 1) * FD],
                                start=(d == ds[0]), stop=(d == ds[-1]))
                        if wave == 0:
                            half = tmpC.tile([P, FD], F32,
                                             name=f"o1_{tt}_{s2}",
                                             tag=f"o1_{tt}_{s2}", bufs=1)
                            osb1[tt][s2] = half
                            nc.vector.tensor_copy(half[:], ps[:])
                        else:
                            nc.vector.scalar_tensor_tensor(
                                o_sb[:, s2 * FD:(s2 + 1) * FD], ps[:], 1.0,
                                osb1[tt][s2][:], op0=OP.mult, op1=OP.add)
                    if wave == 1:
                        nc.sync.dma_start(out.ap()[tt * P:(tt + 1) * P, :],
                                          o_sb[:])

            # ---------- attention (head pairs, chunked exp) -------
            for et in range(ND):
                hA, hB = 2 * et, 2 * et + 1
                ctx_a = ps2.tile([DH + 1, LQ], F32, name="ctxa", tag="ctx",
                                 bufs=2)
                ctx_b = ps2.tile([DH + 1, LQ], F32, name="ctxb", tag="ctx",
                                 bufs=2)
                kA = krT[0:DH, et, :]
                kB = krT[DH:P, et, :]
                qA = qrT[0:DH, et, :]
                qB = qrT[DH:P, et, :]
                for g in range(NT // 2):
                    st0, st1 = 2 * g, 2 * g + 1
                    spsA = ps2.tile([P, 2, LQ], F32, name="spsA",
                                    tag="sps", bufs=2)
                    spsB = ps2.tile([P, 2, LQ], F32, name="spsB",
                                    tag="sps", bufs=2)
                    nc.tensor.matmul(spsA[:, 0, :],
                                     kA[:, st0 * P:(st0 + 1) * P], qA,
                                     start=True, stop=True)
                    nc.tensor.matmul(spsB[:, 0, :],
                                     kB[:, st0 * P:(st0 + 1) * P], qB,
                                     start=True, stop=True)
                    nc.tensor.matmul(spsA[:, 1, :],
                                     kA[:, st1 * P:(st1 + 1) * P], qA,
                                     start=True, stop=True)
                    nc.tensor.matmul(spsB[:, 1, :],
                                     kB[:, st1 * P:(st1 + 1) * P], qB,
                                     start=True, stop=True)
                    expA = tmpC.tile([P, 2, LQ], BF, name="expA",
                                     tag="exp", bufs=8)
                    expB = tmpC.tile([P, 2, LQ], BF, name="expB",
                                     tag="exp", bufs=8)
                    nc.scalar.activation(expA[:], spsA[:], AF.Exp)
                    nc.scalar.activation(expB[:], spsB[:], AF.Exp)
                    for j, st in ((0, st0), (1, st1)):
                        nc.tensor.matmul(ctx_a[:], v_sb[:, st, hA, :],
                                         expA[:, j, :],
                                         start=(st == 0),
                                         stop=(st == NT - 1))
                        nc.tensor.matmul(ctx_b[:], v_sb[:, st, hB, :],
                                         expB[:, j, :],
                                         start=(st == 0),
                                         stop=(st == NT - 1))
                for hh, cps in ((hA, ctx_a), (hB, ctx_b)):
                    half = (hh % 2) * DH
                    rrow = stat.tile([1, LQ], BF, name="rrow", tag="rrow",
                                     bufs=2)
                    with nc.allow_low_precision(reason="softmax denom"):
                        nc.vector.reciprocal(rrow[:], cps[DH:DH + 1, :])
                    rb_ps = ps2.tile([DH, LQ], F32, name="rbps", tag="rbps",
                                     bufs=2)
                    nc.tensor.matmul(rb_ps[:], ones_row[0:1, 0:DH], rrow[:],
                                     start=True, stop=True)
                    rb = tmpC.tile([DH, LQ], BF, name="rb", tag="rb",
                                   bufs=2)
                    nc.vector.tensor_copy(rb[:], rb_ps[:])
                    nc.vector.tensor_mul(ctxT[half:half + DH, et, :],
                                         cps[0:DH, :], rb[:])
                if et == ND // 2 - 1:
                    # low priority: PE picks these up in exp-wait gaps
                    with tc.high_priority(offset=-1000000):
                        out_wave(0)

            # ---------- output projection, wave 2 ----------
            out_wave(1)


_NC_CACHE = {}


def build_nc(do_compile=True, with_c0=False):
    nc = bacc.Bacc("TRN2", target_bir_lowering=False, debug=False,
                   num_devices=NCORES)
    _emit(nc, with_c0)
    if do_compile:
        nc.compile()
    return nc


def _get_nc(with_c0=False):
    if with_c0 not in _NC_CACHE:
        _NC_CACHE[with_c0] = build_nc(do_compile=True, with_c0=with_c0)
    return _NC_CACHE[with_c0]


def _build_tables():
    inv_freq = 1.0 / (ROPE_BASE ** (np.arange(0, DH, 2, dtype=np.float32) / DH))
    t = np.arange(L, dtype=np.float32)
    freqs = np.outer(t, inv_freq)                       # [L, 32]
    cos = np.concatenate([np.cos(freqs)] * 2, axis=1)   # [L, 64]
    sin = np.concatenate([np.sin(freqs)] * 2, axis=1)
    sign = np.where(np.arange(DH) < DH // 2, -1.0, 1.0).astype(np.float32)
    return (cos.astype(ml_dtypes.bfloat16),
            (sin * sign[None, :]).astype(ml_dtypes.bfloat16))


def make_in_maps(x, ln_w, ln_b, w_qkv, q_ln_w, k_ln_w, w_out):
    w_qkv = np.asarray(w_qkv, np.float32)
    ln_w = np.asarray(ln_w, np.float32)
    ln_b = np.asarray(ln_b, np.float32)
    # fold the x-layernorm affine into the projection (exact):
    #   qkv = ((x-mu)*r * ln_w + ln_b) @ W^T
    #       = ((x-mu)*r) @ (W*ln_w)^T + (W @ ln_b)
    wf = w_qkv * ln_w[None, :]
    c0 = (w_qkv @ ln_b).reshape(3, D)
    with_c0 = bool(np.any(c0 != 0.0))
    wqkvT = np.ascontiguousarray(wf.T).astype(ml_dtypes.bfloat16)
    woutT = np.ascontiguousarray(np.asarray(w_out, np.float32).T).astype(
        ml_dtypes.bfloat16)
    cos_t, sin_t = _build_tables()
    x = np.asarray(x, np.float32)
    in_maps = []
    for c in range(NCORES):
        b, s = c // 4, c % 4
        xq = x[b, s * LQ:(s + 1) * LQ]
        xT = np.ascontiguousarray(xq.T).astype(ml_dtypes.bfloat16)
        im = {
            "xT": xT, "wqkvT": wqkvT, "woutT": woutT,
            "q_ln_w": np.asarray(q_ln_w, np.float32).astype(ml_dtypes.bfloat16),
            "k_ln_w": np.asarray(k_ln_w, np.float32).astype(ml_dtypes.bfloat16),
            "cos_t": np.ascontiguousarray(cos_t[s * LQ:(s + 1) * LQ]),
            "sin_t": np.ascontiguousarray(sin_t[s * LQ:(s + 1) * LQ]),
        }
        if with_c0:
            im["c0_t"] = c0.astype(ml_dtypes.bfloat16)
        else:
            im["srow_t"] = wf.sum(axis=1).reshape(3, D).astype(
                ml_dtypes.bfloat16)
        in_maps.append(im)
    return in_maps, with_c0


def kernel(x, ln_w, ln_b, w_qkv, q_ln_w, k_ln_w, w_out, **run_kwargs):
    in_maps, with_c0 = make_in_maps(x, ln_w, ln_b, w_qkv, q_ln_w, k_ln_w,
                                    w_out)
    nc = _get_nc(with_c0)
    res = run_bass_kernel_spmd(nc, in_maps, core_ids=list(range(NCORES)),
                               **run_kwargs)
    out = np.zeros((B, L, D), np.float32)
    for c in range(NCORES):
        b, s = c // 4, c % 4
        out[b, s * LQ:(s + 1) * LQ, :] = res.results[c]["out"]
    return out


# revision 42
# speedup vs baseline: 19.3669x; 17.0489x over previous
"""Trainium2 Bass kernel: fused MHA block (LN -> QKV -> q/k per-token LN ->
RoPE -> SDPA -> out-proj), SPMD over 8 NeuronCores.

Sharding (v3): core c handles batch b = c//4 and token quarter s = c%4 in
GLOBAL token order. Each core projects q/k/v only for its own 512-token
quarter (no cross-core redundancy), then the 4 cores of a batch AllGather
the roped keys and the values; every core runs attention for its 512
queries over all 2048 keys. Host concatenates 8 [512, 1024] output slices.

Design notes (all matmuls bf16 with f32 PSUM accumulation):
  - ln_w folded into w_qkv on the host (W' = W * ln_w); nonzero ln_b enters
    as one K=1 ones-matmul accumulate (c0 = W @ ln_b) per projection half.
  - x normalized IN PLACE in the e-major x slab tile: stats via ones(1/D)
    matmuls; r and mu*r rows broadcast across partitions with K=1 matmuls
    into PSUM (no DRAM bounce).
  - per-token q/k LN: bn_stats on DVE, affine applied on ACT via per-token
    scale/bias pointers (in place); RoPE sin-mul on DVE, cos-mul + add on
    Pool.
  - AllGather #1: roped k quarter [512, D] -> [2048, D], then 4 XBAR
    transposes into feature-major krT. AllGather #2: v quarter -> v_sb
    (strided loads add the fused softmax-denominator ones column).
  - attention: scoresT = krT.T @ qrT per head pair, exp on ACT (the binding
    engine); AV ones-column produces the softmax denominator; denominator
    reciprocal broadcast via K=1 matmul. PSUM evacuations on DVE so ACT
    does nothing but exp.
  - transposes (XBAR) from SP, weight DMAs split gpsimd/sync.
"""

import numpy as np
import ml_dtypes

import concourse.bass as bass
import concourse.mybir as mybir
import concourse.tile as tile
from concourse import bacc
from concourse.bass_utils import run_bass_kernel_spmd

B, L, D, H, DH = 2, 2048, 1024, 16, 64
EPS = 1e-5
ROPE_BASE = 10000.0
NCORES = 8
LQ = L // 4
P = 128
ND = D // P      # 8 feature tiles of 128
NT = L // P      # 16 key token tiles
NTQ = LQ // P    # 4 local token tiles
FD = 512         # psum bank free size (f32)
NSL = L // FD    # 4 key slabs of 512
BF = mybir.dt.bfloat16
F32 = mybir.dt.float32
AF = mybir.ActivationFunctionType
OP = mybir.AluOpType
GROUPS = [[0, 1, 2, 3], [4, 5, 6, 7]]


def _bc_part(ap, parts):
    """Partition-broadcast (step 0) of a [1, ...] DRAM AP to `parts` rows."""
    return bass.AP(tensor=ap.tensor, offset=ap.offset,
                   ap=[[0, parts]] + list(ap.ap[1:]))


def _bc_heads(ap2, n, at=1):
    """Insert a step-0 dim of size n at free position `at` of a 2D sbuf AP."""
    dims = list(ap2.ap)
    return bass.AP(tensor=ap2.tensor, offset=ap2.offset,
                   ap=dims[:at] + [[0, n]] + dims[at:])


def _emit(nc, with_c0):
    xT = nc.dram_tensor("xT", [D, LQ], BF, kind="ExternalInput")
    wqkvT = nc.dram_tensor("wqkvT", [D, 3 * D], BF, kind="ExternalInput")
    woutT = nc.dram_tensor("woutT", [D, D], BF, kind="ExternalInput")
    c0_t = (nc.dram_tensor("c0_t", [3, D], BF, kind="ExternalInput")
            if with_c0 else None)
    srow_t = (None if with_c0 else
              nc.dram_tensor("srow_t", [3, D], BF, kind="ExternalInput"))
    q_ln_w = nc.dram_tensor("q_ln_w", [D], BF, kind="ExternalInput")
    k_ln_w = nc.dram_tensor("k_ln_w", [D], BF, kind="ExternalInput")
    cos_t = nc.dram_tensor("cos_t", [LQ, DH], BF, kind="ExternalInput")
    sin_t = nc.dram_tensor("sin_t", [LQ, DH], BF, kind="ExternalInput")
    out = nc.dram_tensor("out", [LQ, D], F32, kind="ExternalOutput")

    with tile.TileContext(nc) as tc:
        _body(nc, tc, xT, wqkvT, woutT, c0_t, srow_t, q_ln_w, k_ln_w,
              cos_t, sin_t, out)
    return nc


def _rstd_refine(nc, pool, r, vareps, shape, name):
    """One Newton step for r ~= rsqrt(varep): r' = r*(1.5 - 0.5*varep*r^2).
    Guards against ACT sqrt LUT error on hardware. In-place on r."""
    t = pool.tile(list(shape), F32, name=f"{name}_nt", tag=f"{name}_nt", bufs=2)
    nc.scalar.activation(t[:], r[:], AF.Square)
    nc.vector.tensor_mul(t[:], t[:], vareps[:])
    nc.vector.tensor_scalar(t[:], t[:], -0.5, 1.5, op0=OP.mult, op1=OP.add)
    nc.vector.tensor_mul(r[:], r[:], t[:])


def _body(nc, tc, xT, wqkvT, woutT, c0_t, srow_t, q_ln_w, k_ln_w,
          cos_t, sin_t, out):
    import contextlib
    ap_xT = xT.ap().rearrange("(nd p) t -> p nd t", p=P)
    ap_wqkvT = wqkvT.ap().rearrange("(nd p) e -> p nd e", p=P)
    ap_woutT = woutT.ap().rearrange("(nd p) e -> p nd e", p=P)
    ap_cos = cos_t.ap().rearrange("(tt p) j -> p tt j", p=P)
    ap_sin = sin_t.ap().rearrange("(tt p) j -> p tt j", p=P)

    ctx = contextlib.ExitStack()
    with ctx:
        const = ctx.enter_context(tc.tile_pool(name="const", bufs=1))
        wpool = ctx.enter_context(tc.tile_pool(name="wp", bufs=1))
        live = ctx.enter_context(tc.tile_pool(name="live", bufs=1))
        stat = ctx.enter_context(tc.tile_pool(name="stat", bufs=1))
        dram = ctx.enter_context(tc.tile_pool(name="dram", bufs=1, space="DRAM"))

        # ---------- weights first (wk on sync: needed soonest) -------------
        wk_sb = wpool.tile([P, ND, D], BF, name="wk", tag="w1", bufs=1)
        nc.sync.dma_start(wk_sb[:], ap_wqkvT[:, :, D:2 * D])
        wq_sb = wpool.tile([P, ND, D], BF, name="wq", tag="w3", bufs=1)
        nc.gpsimd.dma_start(wq_sb[:], ap_wqkvT[:, :, 0:D])
        wv_sb = wpool.tile([P, ND, D], BF, name="wv", tag="w2", bufs=1)
        nc.gpsimd.dma_start(wv_sb[:], ap_wqkvT[:, :, 2 * D:3 * D])

        # ---------- constants ----------
        qw_sb = const.tile([P, D], BF)      # q_ln_w broadcast to all partitions
        nc.gpsimd.dma_start(qw_sb[:], _bc_part(q_ln_w.ap()[None, :], P))
        kw_sb = const.tile([P, D], BF)
        nc.gpsimd.dma_start(kw_sb[:], _bc_part(k_ln_w.ap()[None, :], P))
        cos_sb = const.tile([P, NTQ, DH], BF)
        nc.gpsimd.dma_start(cos_sb[:], ap_cos)
        sin_sb = const.tile([P, NTQ, DH], BF)
        nc.gpsimd.dma_start(sin_sb[:], ap_sin)
        if c0_t is not None:
            c0_sb = const.tile([1, 3, D], BF)
            nc.gpsimd.dma_start(c0_sb[:], c0_t.ap()[None, :, :])
        if srow_t is not None:
            # colsums of W' per kind, broadcast to all partitions (the
            # raw-x projection's rank-1 mean correction)
            srow_sb = const.tile([P, 3, D], BF)
            nc.gpsimd.dma_start(srow_sb[:], _bc_part(srow_t.ap()[None], P))
        onesD_sb = const.tile([P, 1], BF)    # 1/D column for the stats matmul
        nc.vector.memset(onesD_sb[:], 1.0 / D)
        ones_row = const.tile([1, P], BF)
        nc.vector.memset(ones_row[:], 1.0)

        # ---------- long-lived tensors ----------
        v_sb = live.tile([P, NT, H, DH + 1], BF)
        nc.vector.memset(v_sb[:, :, :, DH:DH + 1], 1.0)
        krT = live.tile([P, ND, L], BF)
        qrT = live.tile([P, ND, LQ], BF)
        ctxT = live.tile([P, ND, LQ], BF)
        xsl = live.tile([P, ND, FD], BF)     # this quarter's x, e-major

        qr_d = dram.tile([LQ, D], BF, bufs=1)
        kq_d = dram.tile([LQ, D], BF, bufs=1)
        vq_d = dram.tile([LQ, D], BF, bufs=1)
        kg_d = dram.tile([L, D], BF, bufs=1)
        vg_d = dram.tile([L, D], BF, bufs=1)

        with tc.tile_pool(name="tmpA", bufs=1) as tmpA, \
             tc.tile_pool(name="ps1", bufs=1, space="PSUM") as ps1:

            # ---------- per-tile helpers ----------
            def project_tile(w_tile, jt, kind):
                """[128 tok, 1024] projection psum pair for local token tile
                jt. kind: 0=q, 1=k, 2=v (selects the folded-ln_b c0 row)."""
                pss = []
                for s2 in range(2):
                    ps = ps1.tile([P, FD], F32, name=f"pj{s2}",
                                  tag=f"pj{s2}", bufs=2)
                    for d in range(ND):
                        nc.tensor.matmul(ps[:],
                                         xsl[:, d, jt * P:(jt + 1) * P],
                                         w_tile[:, d, s2 * FD:(s2 + 1) * FD],
                                         start=(d == 0),
                                         stop=(c0_t is None and d == ND - 1))
                    if c0_t is not None:
                        nc.tensor.matmul(
                            ps[:], ones_row[0:1, :],
                            c0_sb[0:1, kind, s2 * FD:(s2 + 1) * FD],
                            start=False, stop=True)
                    pss.append(ps)
                return pss

            def evac_qk(pss, kind, jt, name):
                """PSUM -> raw. Fast path: raw' = mu*s - ps (the NEGATED
                pre-LN row; the per-token LN absorbs the sign via a negated
                scale pointer and is invariant to the missing 1/r)."""
                raw = tmpA.tile([P, D], BF, name=f"{name}raw", tag="raw",
                                bufs=3)
                if c0_t is None:
                    mu_col = rmu_sb[:, jt, 1:2]
                    with nc.allow_low_precision(reason="pre-LN row to bf16"):
                        for s2 in range(2):
                            sl = slice(s2 * FD, (s2 + 1) * FD)
                            nc.vector.scalar_tensor_tensor(
                                raw[:, sl], srow_sb[:, kind, sl], mu_col,
                                pss[s2][:], op0=OP.mult, op1=OP.subtract)
                else:
                    nc.scalar.copy(raw[:, 0:FD], pss[0][:])
                    nc.scalar.copy(raw[:, FD:D], pss[1][:])
                return raw

            def token_ln_rope(raw, w_row, scale, jt, dst_d, name):
                st6 = stat.tile([P, 2, 6], F32, name=f"{name}bs", tag="bs",
                                bufs=4)
                seg = raw[:].rearrange("p (s f) -> p s f", s=2)
                for s2 in range(2):
                    nc.vector.bn_stats(st6[:, s2, :], seg[:, s2, :])
                mv = stat.tile([P, 2], F32, name=f"{name}mv", tag="mv", bufs=4)
                nc.vector.bn_aggr(mv[:], st6[:])
                vep = stat.tile([P, 1], F32, name=f"{name}ve", tag="ve",
                                bufs=4)
                nc.vector.tensor_scalar(vep[:], mv[:, 1:2], 1.0, EPS,
                                        op0=OP.mult, op1=OP.add)
                r = stat.tile([P, 1], F32, name=f"{name}r", tag="lr", bufs=4)
                nc.scalar.activation(r[:], vep[:], AF.Sqrt)
                nc.vector.reciprocal(r[:], r[:])
                _rstd_refine(nc, stat, r, vep, (P, 1), "t")
                # fast path: raw is negated -> negate the LN scale (exact)
                rs = -scale if c0_t is None else scale
                if rs != 1.0:
                    nc.vector.tensor_scalar_mul(r[:], r[:], rs)
                nmb = stat.tile([P, 1], F32, name=f"{name}nmb", tag="nmb",
                                bufs=4)
                nc.vector.tensor_scalar(nmb[:], mv[:, 0:1], r[:], -1.0,
                                        op0=OP.mult, op1=OP.mult)
                nc.scalar.activation(raw[:], raw[:], AF.Identity,
                                     bias=nmb[:], scale=r[:])
                nc.vector.tensor_mul(raw[:], raw[:], w_row[:])
                xn = raw[:].rearrange("p (h j) -> p h j", j=DH)
                t2 = tmpA.tile([P, H, DH], BF, name=f"{name}t2", tag="rp2",
                               bufs=2)
                nc.vector.tensor_mul(t2[:, :, 0:DH // 2],
                                     xn[:, :, DH // 2:DH],
                                     _bc_heads(sin_sb[:, jt, 0:DH // 2], H))
                nc.vector.tensor_mul(t2[:, :, DH // 2:DH],
                                     xn[:, :, 0:DH // 2],
                                     _bc_heads(sin_sb[:, jt, DH // 2:DH], H))
                t3 = tmpA.tile([P, H, DH], BF, name=f"{name}t3", tag="rp3",
                               bufs=2)
                nc.gpsimd.tensor_mul(t3[:], xn,
                                     _bc_heads(cos_sb[:, jt, :], H))
                nc.gpsimd.tensor_add(t3[:], t3[:], t2[:])
                nc.sync.dma_start(dst_d[jt * P:(jt + 1) * P, :],
                                  t3[:].rearrange("p h j -> p (h j)"))

            # ---------- phase 1: stats + in-place LN (one local slab) ------
            nc.sync.dma_start(xsl[:], ap_xT)
            ps_s = ps1.tile([1, FD], F32, name="xs", tag="xs", bufs=1)
            ps_q = ps1.tile([1, FD], F32, name="xss", tag="xss", bufs=1)
            for d in range(ND):
                sq = tmpA.tile([P, FD], BF, name="xsq", tag="xsq", bufs=2)
                nc.scalar.activation(sq[:], xsl[:, d, :], AF.Square)
                nc.tensor.matmul(ps_s[:], onesD_sb[:], xsl[:, d, :],
                                 start=(d == 0), stop=(d == ND - 1))
                nc.tensor.matmul(ps_q[:], onesD_sb[:], sq[:],
                                 start=(d == 0), stop=(d == ND - 1))
            # ps_s = mean, ps_q = E[x^2]
            vep = stat.tile([1, FD], F32, name="xvep", tag="xvep")
            nc.scalar.activation(vep[:], ps_s[:], AF.Square)
            nc.vector.tensor_scalar(vep[:], vep[:], -1.0, EPS,
                                    op0=OP.mult, op1=OP.add)
            nc.vector.scalar_tensor_tensor(vep[:], ps_q[:], 1.0, vep[:],
                                           op0=OP.mult, op1=OP.add)
            r = stat.tile([1, FD], F32, name="xr", tag="xr")
            nc.scalar.activation(r[:], vep[:], AF.Sqrt)
            nc.vector.reciprocal(r[:], r[:])
            _rstd_refine(nc, stat, r, vep, (1, FD), "x")
            if c0_t is None:
                # token-major [-r | mu] columns via tiny K=1 matmuls; x
                # stays RAW (projections start without waiting for stats)
                rows = stat.tile([1, 2, FD], BF, name="xrows", tag="xrows",
                                 bufs=1)
                with nc.allow_low_precision(reason="stat rows to bf16"):
                    nc.vector.tensor_scalar_mul(rows[:, 0, :], r[:], -1.0)
                    nc.vector.tensor_copy(rows[:, 1, :], ps_s[:])
                rmu_ps = ps1.tile([P, NTQ, 2], F32, name="rmu", tag="rmu",
                                  bufs=1)
                for jt in range(NTQ):
                    for q2 in range(2):
                        nc.tensor.matmul(
                            rmu_ps[:, jt, q2:q2 + 1],
                            rows[0:1, q2, jt * P:(jt + 1) * P],
                            ones_row[0:1, 0:1], start=True, stop=True)
                rmu_sb = stat.tile([P, NTQ, 2], F32, name="rmusb",
                                   tag="rmusb", bufs=1)
                nc.vector.tensor_copy(rmu_sb[:], rmu_ps[:])
            else:
                rows = stat.tile([1, 2, FD], BF, name="xrows", tag="xrows",
                                 bufs=2)
                nc.vector.tensor_copy(rows[:, 0, :], r[:])
                with nc.allow_low_precision(reason="mu*r row to bf16"):
                    nc.vector.tensor_mul(rows[:, 1, :], ps_s[:], r[:])
                bc_ps = ps1.tile([P, 2, FD], F32, name="bc", tag="bc",
                                 bufs=1)
                nc.tensor.matmul(bc_ps[:, 0, :], ones_row[0:1, :],
                                 rows[:, 0, :], start=True, stop=True)
                nc.tensor.matmul(bc_ps[:, 1, :], ones_row[0:1, :],
                                 rows[:, 1, :], start=True, stop=True)
                rbmr = tmpA.tile([P, 2, FD], BF, name="rbmr", tag="rbmr",
                                 bufs=1)
                nc.scalar.copy(rbmr[:], bc_ps[:])
                # in-place: x <- x*r - mu*r  (e-major)
                for d in range(ND):
                    nc.vector.tensor_mul(xsl[:, d, :], xsl[:, d, :],
                                         rbmr[:, 0, :])
                    nc.gpsimd.tensor_sub(xsl[:, d, :], xsl[:, d, :],
                                         rbmr[:, 1, :])

            # ---------- local k quarter -> AllGather (gpsimd) ----------
            for jt in range(NTQ):
                pss = project_tile(wk_sb, jt, 1)
                raw = evac_qk(pss, 1, jt, "k")
                token_ln_rope(raw, kw_sb, 1.0, jt, kq_d, "k")
            # NOTE: the sim cost model charges collectives ~120us to the
            # issuing engine queue; real HW is a doorbell + ~15us (4-rank
            # 1MB AllGather). Engines are picked so nothing
            # latency-critical sits behind a collective in its queue.
            nc.gpsimd.collective_compute(
                "AllGather", OP.bypass, replica_groups=GROUPS,
                ins=[kq_d[:].opt()], outs=[kg_d[:].opt()])

            # ---------- local q quarter -> qrT (transpose early: it gates
            # the first scores together with krT slab 0) ----------
            for jt in range(NTQ):
                pss = project_tile(wq_sb, jt, 0)
                raw = evac_qk(pss, 0, jt, "q")
                token_ln_rope(raw, qw_sb, DH ** -0.5, jt, qr_d, "q")
            nc.sync.dma_start_transpose(qrT[:], qr_d[:])

            # ---------- local v quarter ----------
            for jt in range(NTQ):
                pss = project_tile(wv_sb, jt, 2)
                vq_sb = tmpA.tile([P, D], BF, name="vq", tag="vq", bufs=2)
                if c0_t is None:
                    # v = (ps - mu*s_v)*r = (mu*s_v - ps) * (-r)
                    with nc.allow_low_precision(reason="v to bf16"):
                        for s2 in range(2):
                            sl = slice(s2 * FD, (s2 + 1) * FD)
                            vtmp = tmpA.tile([P, FD], BF, name="vtmp",
                                             tag="vtmp", bufs=2)
                            nc.vector.scalar_tensor_tensor(
                                vtmp[:], srow_sb[:, 2, sl],
                                rmu_sb[:, jt, 1:2], pss[s2][:],
                                op0=OP.mult, op1=OP.subtract)
                            nc.vector.tensor_scalar_mul(
                                vq_sb[:, sl], vtmp[:], rmu_sb[:, jt, 0:1])
                else:
                    nc.vector.tensor_copy(vq_sb[:, 0:FD], pss[0][:])
                    nc.vector.tensor_copy(vq_sb[:, FD:D], pss[1][:])
                nc.sync.dma_start(vq_d[jt * P:(jt + 1) * P, :], vq_sb[:])
            nc.gpsimd.collective_compute(
                "AllGather", OP.bypass, replica_groups=GROUPS,
                ins=[vq_d[:].opt()], outs=[vg_d[:].opt()])
            for j in range(NSL):
                nc.sync.dma_start_transpose(
                    krT[:, :, j * FD:(j + 1) * FD],
                    kg_d[j * FD:(j + 1) * FD, :])
            for st in range(NT):
                src = vg_d[st * P:(st + 1) * P, :].rearrange(
                    "p (h e) -> p h e", e=DH)
                nc.sync.dma_start(v_sb[:, st, :, 0:DH], src)

        # wq slot done; load w_out for the final projection (SWDGE).
        wo_sb = wpool.tile([P, ND, D], BF, name="wo", tag="w3", bufs=1)
        nc.gpsimd.dma_start(wo_sb[:], ap_woutT)

        with tc.tile_pool(name="tmpC", bufs=1) as tmpC, \
             tc.tile_pool(name="ps2", bufs=1, space="PSUM") as ps2:
            # Wave-1 of the output projection (contraction d=0..3) runs
            # right after et=3 so its matmuls hide under the ACT-bound
            # attention window; wave-2 accumulates on top after et=7.
            osb1 = [[None] * 2 for _ in range(NTQ)]

            def out_wave(wave):
                ds = range(0, ND // 2) if wave == 0 else range(ND // 2, ND)
                for tt in range(NTQ):
                    o_sb = (None if wave == 0 else
                            tmpC.tile([P, D], F32, name="osb", tag="osb",
                                      bufs=2))
                    for s2 in range(2):
                        ps = ps2.tile([P, FD], F32, name="ops", tag="rbps",
                                      bufs=2)
                        for d in ds:
                            nc.tensor.matmul(
                                ps[:], ctxT[:, d, tt * P:(tt + 1) * P],
                                wo_sb[:, d, s2 * FD:(s2 + 1) * FD],
                                start=(d == ds[0]), stop=(d == ds[-1]))
                        if wave == 0:
                            half = tmpC.tile([P, FD], F32,
                                             name=f"o1_{tt}_{s2}",
                                             tag=f"o1_{tt}_{s2}", bufs=1)
                            osb1[tt][s2] = half
                            nc.vector.tensor_copy(half[:], ps[:])
                        else:
                            nc.vector.scalar_tensor_tensor(
                                o_sb[:, s2 * FD:(s2 + 1) * FD], ps[:], 1.0,
                                osb1[tt][s2][:], op0=OP.mult, op1=OP.add)
                    if wave == 1:
                        nc.sync.dma_start(out.ap()[tt * P:(tt + 1) * P, :],
                                          o_sb[:])

            # ---------- attention (head pairs, chunked exp) -------
            for et in range(ND):
                hA, hB = 2 * et, 2 * et + 1
                ctx_a = ps2.tile([P, LQ], F32, name="ctxa", tag="ctx",
                                 bufs=2)
                ctx_b = ps2.tile([P, LQ], F32, name="ctxb", tag="ctx",
                                 bufs=2)
                kA = krT[0:DH, et, :]
                kB = krT[DH:P, et, :]
                qA = qrT[0:DH, et, :]
                qB = qrT[DH:P, et, :]
                for g in range(NT // 2):
                    st0, st1 = 2 * g, 2 * g + 1
                    spsA = ps2.tile([P, 2, LQ], F32, name="spsA",
                                    tag="sps", bufs=2)
                    spsB = ps2.tile([P, 2, LQ], F32, name="spsB",
                                    tag="sps", bufs=2)
                    nc.tensor.matmul(spsA[:, 0, :],
                                     kA[:, st0 * P:(st0 + 1) * P], qA,
                                     start=True, stop=True)
                    nc.tensor.matmul(spsB[:, 0, :],
                                     kB[:, st0 * P:(st0 + 1) * P], qB,
                                     start=True, stop=True)
                    nc.tensor.matmul(spsA[:, 1, :],
                                     kA[:, st1 * P:(st1 + 1) * P], qA,
                                     start=True, stop=True)
                    nc.tensor.matmul(spsB[:, 1, :],
                                     kB[:, st1 * P:(st1 + 1) * P], qB,
                                     start=True, stop=True)
                    expA = tmpC.tile([P, 2, LQ], BF, name="expA",
                                     tag="exp", bufs=8)
                    expB = tmpC.tile([P, 2, LQ], BF, name="expB",
                                     tag="exp", bufs=8)
                    nc.scalar.activation(expA[:], spsA[:], AF.Exp)
                    nc.scalar.activation(expB[:], spsB[:], AF.Exp)
                    for j, st in ((0, st0), (1, st1)):
                        nc.tensor.matmul(ctx_a[0:DH + 1, :],
                                         v_sb[:, st, hA, :],
                                         expA[:, j, :],
                                         start=(st == 0),
                                         stop=(st == NT - 1))
                        nc.tensor.matmul(ctx_b[0:DH + 1, :],
                                         v_sb[:, st, hB, :],
                                         expB[:, j, :],
                                         start=(st == 0),
                                         stop=(st == NT - 1))
                for hh, cps in ((hA, ctx_a), (hB, ctx_b)):
                    half = (hh % 2) * DH
                    rrow = stat.tile([1, LQ], BF, name="rrow", tag="rrow",
                                     bufs=2)
                    with nc.allow_low_precision(reason="softmax denom"):
                        nc.vector.reciprocal(rrow[:], cps[DH:DH + 1, :])
                    # broadcast 1/den into rows 64..127 of this ctx tile's
                    # own PSUM bank (frees the rbps slots for the out-proj)
                    nc.tensor.matmul(cps[DH:P, :], ones_row[0:1, 0:DH],
                                     rrow[:], start=True, stop=True)
                    rb = tmpC.tile([DH, LQ], BF, name="rb", tag="rb",
                                   bufs=2)
                    nc.vector.tensor_copy(rb[:], cps[DH:P, :])
                    nc.vector.tensor_mul(ctxT[half:half + DH, et, :],
                                         cps[0:DH, :], rb[:])
                if et == ND // 2 - 1:
                    # low priority: PE picks these up in exp-wait gaps
                    with tc.high_priority(offset=-1000000):
                        out_wave(0)

            # ---------- output projection, wave 2 ----------
            out_wave(1)


_NC_CACHE = {}


def build_nc(do_compile=True, with_c0=False):
    nc = bacc.Bacc("TRN2", target_bir_lowering=False, debug=False,
                   num_devices=NCORES)
    _emit(nc, with_c0)
    if do_compile:
        nc.compile()
    return nc


def _get_nc(with_c0=False):
    if with_c0 not in _NC_CACHE:
        _NC_CACHE[with_c0] = build_nc(do_compile=True, with_c0=with_c0)
    return _NC_CACHE[with_c0]


def _build_tables():
    inv_freq = 1.0 / (ROPE_BASE ** (np.arange(0, DH, 2, dtype=np.float32) / DH))
    t = np.arange(L, dtype=np.float32)
    freqs = np.outer(t, inv_freq)                       # [L, 32]
    cos = np.concatenate([np.cos(freqs)] * 2, axis=1)   # [L, 64]
    sin = np.concatenate([np.sin(freqs)] * 2, axis=1)
    sign = np.where(np.arange(DH) < DH // 2, -1.0, 1.0).astype(np.float32)
    return (cos.astype(ml_dtypes.bfloat16),
            (sin * sign[None, :]).astype(ml_dtypes.bfloat16))


def make_in_maps(x, ln_w, ln_b, w_qkv, q_ln_w, k_ln_w, w_out):
    w_qkv = np.asarray(w_qkv, np.float32)
    ln_w = np.asarray(ln_w, np.float32)
    ln_b = np.asarray(ln_b, np.float32)
    # fold the x-layernorm affine into the projection (exact):
    #   qkv = ((x-mu)*r * ln_w + ln_b) @ W^T
    #       = ((x-mu)*r) @ (W*ln_w)^T + (W @ ln_b)
    wf = w_qkv * ln_w[None, :]
    c0 = (w_qkv @ ln_b).reshape(3, D)
    with_c0 = bool(np.any(c0 != 0.0))
    wqkvT = np.ascontiguousarray(wf.T).astype(ml_dtypes.bfloat16)
    woutT = np.ascontiguousarray(np.asarray(w_out, np.float32).T).astype(
        ml_dtypes.bfloat16)
    cos_t, sin_t = _build_tables()
    x = np.asarray(x, np.float32)
    in_maps = []
    for c in range(NCORES):
        b, s = c // 4, c % 4
        xq = x[b, s * LQ:(s + 1) * LQ]
        xT = np.ascontiguousarray(xq.T).astype(ml_dtypes.bfloat16)
        im = {
            "xT": xT, "wqkvT": wqkvT, "woutT": woutT,
            "q_ln_w": np.asarray(q_ln_w, np.float32).astype(ml_dtypes.bfloat16),
            "k_ln_w": np.asarray(k_ln_w, np.float32).astype(ml_dtypes.bfloat16),
            "cos_t": np.ascontiguousarray(cos_t[s * LQ:(s + 1) * LQ]),
            "sin_t": np.ascontiguousarray(sin_t[s * LQ:(s + 1) * LQ]),
        }
        if with_c0:
            im["c0_t"] = c0.astype(ml_dtypes.bfloat16)
        else:
            im["srow_t"] = wf.sum(axis=1).reshape(3, D).astype(
                ml_dtypes.bfloat16)
        in_maps.append(im)
    return in_maps, with_c0


def kernel(x, ln_w, ln_b, w_qkv, q_ln_w, k_ln_w, w_out, **run_kwargs):
    in_maps, with_c0 = make_in_maps(x, ln_w, ln_b, w_qkv, q_ln_w, k_ln_w,
                                    w_out)
    nc = _get_nc(with_c0)
    res = run_bass_kernel_spmd(nc, in_maps, core_ids=list(range(NCORES)),
                               **run_kwargs)
    out = np.zeros((B, L, D), np.float32)
    for c in range(NCORES):
        b, s = c // 4, c % 4
        out[b, s * LQ:(s + 1) * LQ, :] = res.results[c]["out"]
    return out
